# revision 16
# baseline (speedup 1.0000x reference)
"""nn_AttentionPool Trainium2 kernel (fp8-DoubleRow MLP + fp16 scatter).

kernel(x, batch, W1, b1, W2, b2) -> np.ndarray [2048, 1024] float32

Strategy (8 NeuronCores, SPMD, data-parallel over node rows; batch is
sorted so each core covers a contiguous segment range):
  - Host ships per core: x twice — dim-major fp8-e4m3 (MLP moving
    operand, DoubleRow-paired over the 1024 contraction) and node-major
    fp16 (scatter moving operand; fp16 because scatter errors hit the
    output directly).
  - Per 4-tile group (512 nodes) on device:
      PE:  hT[hh] += W1[:,c,:,hh]^T @ xT[:,c]   (fp8 DoubleRow, K=256/mm)
      ACT: thT = tanh(hT + b1)                  (fp16 out)
      PE:  s[i] += thT_slice^T @ w2             (fp16, N=1, FWL loads)
      DVE: sb = s - 25     (bias for the masked exp)
      DVE: mask[i] = (iota == rel)*(b2+25)      (0 or b2+25)
      ACT: A[i] = exp(mask + sb) -> fp16 one-hot row weights
           (match -> exp(s+b2), miss -> exp(s-25) ~ 0 in fp16)
      DVE: e[:,t] = rowsum(A)  (exactly the weights used in the scatter)
      PE:  u_win += A^T @ x    (fp16, 2x N=512, into static node-window
           PSUM accumulators; 3 windows of ~86 tiles per core)
  - Deep software pipeline (score 2 groups behind the MLP, scatter 4
    behind) so the tanh->s->exp cross-engine chain never starves the PE.
  - Host: accumulates window outputs by true segment base, builds
    denominators from e, divides (reference epsilon semantics).
Max-shift note: s in [-1.2, 1.2] for this model so unshifted exp is safe;
softmax normalization cancels any constant shift.
"""
import os
import sys
import types

import numpy as np

P = 128
DIM = 1024
HID = 256
GRP = 4            # 128-node tiles per group
N_CORES = 8
NUM_SEG = 2048
NCH = 4            # DoubleRow contraction chunks of 256 over DIM

# ---------------------------------------------------------------------------
# environment compat (axon-tunneled trn2 + this walrus build)
# ---------------------------------------------------------------------------

def _install_ntff_hook():
    """antenv.axon_hooks is absent in this image; reconstruct it so
    trace=True (KERNEL_TRACE=1) can profile. Harmless if unused."""
    if "antenv.axon_hooks" in sys.modules:
        return
    m = types.ModuleType("antenv.axon_hooks")
    m._hook = None
    m.set_axon_ntff_profile_hook = lambda h: setattr(m, "_hook", h)
    m.get_axon_ntff_profile_hook = lambda: m._hook
    sys.modules["antenv.axon_hooks"] = m
    try:
        from trn_agent_boot.trn_boot import _ntff_profile_via_ctypes
        m.set_axon_ntff_profile_hook(
            _ntff_profile_via_ctypes("/opt/axon/libaxon_pjrt.so"))
    except Exception:
        pass


def _install_tile_compat():
    """This walrus accepts at most ONE sem wait per instruction; Tile's exit
    drain carries one per live proc. Patch the drain to spread waits."""
    from concourse import mybir
    from concourse.tile import TileContext, ScopedClock

    if getattr(TileContext, "_attnpool_patched", False):
        return

    def _patched(self, tick_clock, wait_clock):
        drain_inst = self.nc.sync.drain()
        wait_clock.add_sem_waits(
            drain_inst.ins, ScopedClock({None: tick_clock.global_clock}))
        si = drain_inst.ins.sync_info
        waits = list(si.on_wait or [])
        if len(waits) > 1:
            si.on_wait = waits[:1]
            for i, w in enumerate(waits[1:]):
                nop = self.nc.sync.nop(nofuse=True, hint=f"tailwait{i}")
                nop.ins.sync_info = mybir.SyncInfo(on_wait=[w], on_update=[])
        self.nc.all_engine_barrier()
        popped = self.nc._tile_sem_poison_stack.pop()
        assert popped is self._sem_poison
        self.nc.clear_and_free_semaphores(list(self.sems.allocated().values()))
        self.nc.all_engine_barrier()

    TileContext._drain_and_barrier = _patched
    TileContext._attnpool_patched = True


def _patch_sim_dma_cost():
    """The Tile scheduler chooses instruction order from a cost-model sim.
    Its DMA model (~330GB/s per dma_start, 1.7us init) makes next-group
    x tiles look perpetually not-ready, so the greedy scheduler collapses
    the software pipeline into a serial per-group chain (PE idles ~1.5us
    per group waiting on the tanh->s->exp chain). Model DMA as 4x faster
    for scheduling only — real DMAs are deeply double-buffered, so the
    emitted lags then survive into the schedule. HW execution is
    unaffected (this spec feeds only the scheduling sim)."""
    from concourse import hw_specs
    if not getattr(hw_specs.TRN2Spec, "_attnpool_dma_patched", False):
        hw_specs.TRN2Spec.DMA_CYCLE = hw_specs.TRN2Spec.DMA_CYCLE / 4.0
        hw_specs.TRN2Spec._attnpool_dma_patched = True


def _split_multi_waits(nc):
    """Post-pass: hoist extra sem waits onto single-wait NOPs."""
    from concourse import mybir
    n = 0
    for f in nc.m.functions:
        for blk in f.blocks:
            new = []
            for inst in blk.instructions:
                si = inst.sync_info
                waits = list(si.on_wait or []) if si else []
                if len(waits) > 1:
                    for w in waits[:-1]:
                        n += 1
                        nop = mybir.InstNoOp(name=f"I-waitsplit{n}", ins=[], outs=[])
                        nop.engine = inst.engine
                        nop.sync_info = mybir.SyncInfo(on_wait=[w], on_update=[])
                        new.append(nop)
                    si.on_wait = waits[-1:]
                new.append(inst)
            blk.instructions = new


# ---------------------------------------------------------------------------
# device program
# ---------------------------------------------------------------------------

def _build_kernel(NT, windows, b2_plus_25):
    """windows: list of (a, b) tile ranges (128-node units)."""
    from concourse import bass, mybir
    import concourse.tile as tile

    f32 = mybir.dt.float32
    fp16 = mybir.dt.float16
    fp8 = mybir.dt.float8e4
    DR = mybir.MatmulPerfMode.DoubleRow

    nc = bass.Bass()
    NW = len(windows)
    NG = NT // GRP

    # x: node-major fp16 for scatter. [g][p][(t, d)], node = g*512+t*128+p
    x_in = nc.declare_dram_parameter("x", [NG, P, GRP * DIM], fp16,
                                     isOutput=False)
    # xT: dim-major fp8 for MLP. [g][p][c(4), i(2), n(512)], d = c*256+i*128+p
    xT_in = nc.declare_dram_parameter("xT", [NG, P, NCH * 2 * GRP * P], fp8,
                                      isOutput=False)
    rel_in = nc.declare_dram_parameter("rel", [P, NT], f32, isOutput=False)
    # w1: [p][c(4), i(2), h(256)] = W1[c*256 + i*128 + p, h]
    w1_in = nc.declare_dram_parameter("w1", [P, NCH * 2 * HID], fp8,
                                      isOutput=False)
    # w2: [p][hh(2)] = W2[hh*128 + p]
    w2_in = nc.declare_dram_parameter("w2", [P, 2], fp16, isOutput=False)
    b1_in = nc.declare_dram_parameter("b1", [P, 2], f32, isOutput=False)
    iota_in = nc.declare_dram_parameter("iota", [P, P], f32, isOutput=False)
    u_out = nc.declare_dram_parameter("u", [NW, P, DIM], f32, isOutput=True)
    e_out = nc.declare_dram_parameter("e", [P, NT], f32, isOutput=True)

    win_start = {a: w for w, (a, b) in enumerate(windows)}
    win_end = {b - 1: w for w, (a, b) in enumerate(windows)}
    tile_win = {}
    for w, (a, b) in enumerate(windows):
        for t in range(a, b):
            tile_win[t] = w

    with tile.TileContext(nc) as tc:
        with tc.tile_pool(name="const", bufs=1) as const, \
             tc.tile_pool(name="xpool", bufs=13) as xpool, \
             tc.tile_pool(name="xtpool", bufs=11) as xtpool, \
             tc.tile_pool(name="thpool", bufs=4) as thpool, \
             tc.tile_pool(name="apool", bufs=16) as apool, \
             tc.tile_pool(name="mpool", bufs=4) as mpool, \
             tc.tile_pool(name="spool", bufs=3) as spool, \
             tc.tile_pool(name="opool", bufs=2) as opool, \
             tc.tile_pool(name="pp_h", bufs=2, space="PSUM") as pp_h, \
             tc.tile_pool(name="pp_s", bufs=2, space="PSUM") as pp_s, \
             tc.tile_pool(name="pp_u", bufs=1, space="PSUM") as pp_u:

            w1t = const.tile([P, NCH, 2, HID], fp8)
            nc.sync.dma_start(out=w1t[:].rearrange("p c i h -> p (c i h)"),
                              in_=w1_in[:])
            w2t = const.tile([P, 2], fp16)
            nc.sync.dma_start(out=w2t[:], in_=w2_in[:])
            b1t = const.tile([P, 2], f32)
            nc.sync.dma_start(out=b1t[:], in_=b1_in[:])
            iota = const.tile([P, P], f32)
            nc.sync.dma_start(out=iota[:], in_=iota_in[:])
            relt = const.tile([P, NT], f32)
            nc.sync.dma_start(out=relt[:], in_=rel_in[:])
            e_stage = const.tile([P, NT], f32)

            state = {}
            ugroups = {}

            def emit_mlp(g):
                t0 = g * GRP
                xTg = xtpool.tile([P, NCH, 2, GRP * P], fp8, tag="xTg")
                nc.sync.dma_start(
                    out=xTg[:].rearrange("p c i n -> p (c i n)"), in_=xT_in[g])
                xg = xpool.tile([P, GRP, DIM], fp16, tag="xg")
                nc.sync.dma_start(
                    out=xg[:].rearrange("p t d -> p (t d)"), in_=x_in[g])
                mg = mpool.tile([P, GRP, P], f32, tag="mg")
                for i in range(GRP):
                    nc.vector.tensor_scalar(
                        out=mg[:, i],
                        in0=iota[:],
                        scalar1=relt[:, t0 + i:t0 + i + 1],
                        scalar2=float(b2_plus_25),
                        op0=mybir.AluOpType.is_equal,
                        op1=mybir.AluOpType.mult)

                hts = []
                for hh in range(2):
                    hTp = pp_h.tile([P, GRP * P], f32, tag=f"hT{hh}",
                                    name=f"hTp{hh}")
                    for c in range(NCH):
                        nc.tensor.matmul(
                            hTp[:],
                            lhsT=w1t[:, c, :, hh * P:(hh + 1) * P],
                            rhs=xTg[:, c],
                            start=(c == 0), stop=(c == NCH - 1),
                            perf_mode=DR)
                    hts.append(hTp)

                thT = thpool.tile([P, 2, GRP * P], fp16, tag="thT")
                for hh in range(2):
                    nc.scalar.activation(
                        thT[:, hh], hts[hh][:],
                        mybir.ActivationFunctionType.Tanh,
                        bias=b1t[:, hh:hh + 1])
                state[g] = {"xg": xg, "thT": thT, "mg": mg}

            def emit_s(g):
                st = state[g]
                sp = pp_s.tile([P, GRP], mybir.dt.float32, tag="sp")
                for i in range(GRP):
                    for hh in range(2):
                        nc.tensor.matmul(
                            sp[:, i:i + 1],
                            lhsT=st["thT"][:, hh, i * P:(i + 1) * P],
                            rhs=w2t[:, hh:hh + 1],
                            start=(hh == 0), stop=(hh == 1))
                sb = spool.tile([P, GRP], mybir.dt.float32, tag="sb")
                nc.vector.tensor_scalar(
                    out=sb[:], in0=sp[:], scalar1=-25.0, scalar2=None,
                    op0=mybir.AluOpType.add, op1=mybir.AluOpType.bypass)
                As = []
                for i in range(GRP):
                    t = g * GRP + i
                    A = apool.tile([P, P], fp16, tag="A")
                    nc.scalar.activation(
                        A[:], st["mg"][:, i],
                        mybir.ActivationFunctionType.Exp,
                        bias=sb[:, i:i + 1])
                    As.append(A)
                    nc.vector.reduce_sum(
                        e_stage[:, t:t + 1], A[:], axis=mybir.AxisListType.X)
                st["As"] = As

            def emit_scatter(g):
                st = state[g]
                for i in range(GRP):
                    t = g * GRP + i
                    xt = st["xg"][:, i]
                    w = tile_win[t]
                    if t in win_start:
                        uwin = pp_u.tile([P, DIM], mybir.dt.float32,
                                         tag="uwin")
                        ugroups[w] = uwin
                    up = ugroups[w]
                    for half in range(2):
                        nc.tensor.matmul(
                            up[:, half * 512:(half + 1) * 512],
                            lhsT=st["As"][i][:],
                            rhs=xt[:, half * 512:(half + 1) * 512],
                            start=(t in win_start), stop=(t in win_end))
                    if t in win_end:
                        us = opool.tile([P, DIM], mybir.dt.float32, tag="us")
                        nc.vector.tensor_copy(us[:, 0:512], up[:, 0:512])
                        nc.scalar.copy(us[:, 512:1024], up[:, 512:1024])
                        nc.sync.dma_start(out=u_out[w], in_=us[:])
                del state[g]

            S_LAG, SC_LAG = 2, 4
            EQ = NT // 4  # e_out flushed in quarters to keep it off the tail
            for g in range(NG):
                emit_mlp(g)
                if g >= S_LAG:
                    emit_s(g - S_LAG)
                    tq = (g - S_LAG + 1) * GRP
                    if tq % EQ == 0:
                        nc.sync.dma_start(out=e_out[:, tq - EQ:tq],
                                          in_=e_stage[:, tq - EQ:tq])
                if g >= SC_LAG:
                    emit_scatter(g - SC_LAG)
            for g in range(NG - S_LAG, NG):
                emit_s(g)
            nc.sync.dma_start(out=e_out[:, NT - EQ:NT],
                              in_=e_stage[:, NT - EQ:NT])
            for g in range(NG - SC_LAG, NG):
                emit_scatter(g)


    return nc


# ---------------------------------------------------------------------------
# host wrapper
# ---------------------------------------------------------------------------

def _make_windows(n, nw):
    base, rem = divmod(n, nw)
    sizes = [base + (1 if i < rem else 0) for i in range(nw)]
    out, a = [], 0
    for s in sizes:
        out.append((a, a + s))
        a += s
    return out


def _reference_numpy(x, batch, W1, b1, W2, b2):
    """Fallback for inputs outside this kernel's structural assumptions."""
    h = np.tanh(x.astype(np.float64) @ W1.astype(np.float64) + b1)
    s = (h @ W2.astype(np.float64) + b2).ravel()
    e = np.exp(s - s.max())
    denom = np.zeros(NUM_SEG, dtype=np.float64)
    np.add.at(denom, batch, e)
    attn = e / (denom[batch] + 1e-8)
    out = np.zeros((NUM_SEG, x.shape[1]), dtype=np.float64)
    np.add.at(out, batch, attn[:, None] * x.astype(np.float64))
    return out.astype(np.float32)


def kernel(x, batch, W1, b1, W2, b2):
    import ml_dtypes
    fp8 = ml_dtypes.float8_e4m3

    x = np.ascontiguousarray(np.asarray(x, dtype=np.float32))
    batch64 = np.asarray(batch).astype(np.int64)
    W1 = np.asarray(W1, dtype=np.float32)
    b1 = np.asarray(b1, dtype=np.float32)
    W2 = np.asarray(W2, dtype=np.float32)
    b2 = np.asarray(b2, dtype=np.float32)

    N = x.shape[0]
    pc = N // N_CORES
    NT = pc // P
    NG = NT // GRP if NT else 0

    ok = (N == N_CORES * pc and pc == NT * P and NT % GRP == 0
          and x.shape[1] == DIM and W1.shape == (DIM, HID)
          and np.all(batch64[:-1] <= batch64[1:])
          and batch64.min() >= 0 and batch64.max() < NUM_SEG)
    if not ok:
        return _reference_numpy(x, batch64, W1, b1, W2, b2)

    windows = _make_windows(NT, 3)
    iota = np.tile(np.arange(P, dtype=np.float32), (P, 1))
    b2f = float(b2.reshape(-1)[0])

    # weights, shared across cores
    w1q = np.ascontiguousarray(
        W1.astype(fp8).reshape(NCH, 2, P, HID).transpose(2, 0, 1, 3)
        .reshape(P, NCH * 2 * HID))
    w2q = np.ascontiguousarray(W2.astype(np.float16).reshape(2, P).T)
    b1q = np.ascontiguousarray(b1.reshape(2, P).T)

    x8 = x.astype(fp8)

    in_maps, meta = [], []
    for c in range(N_CORES):
        bb = batch64[c * pc:(c + 1) * pc]
        rel = np.empty((NT, P), dtype=np.float32)
        bases = []
        for w, (a, b) in enumerate(windows):
            base = int(bb[a * P])
            bases.append(base)
            seg_rel = bb[a * P:b * P] - base
            if seg_rel.min() < 0 or seg_rel.max() >= P:
                return _reference_numpy(x, batch64, W1, b1, W2, b2)
            rel[a:b] = seg_rel.reshape(b - a, P).astype(np.float32)
        # node-major fp16 (scatter)
        x16 = x[c * pc:(c + 1) * pc].astype(np.float16)
        xt = np.ascontiguousarray(
            x16.reshape(NG, GRP, P, DIM).transpose(0, 2, 1, 3)
               .reshape(NG, P, GRP * DIM))
        # dim-major fp8 (MLP): [g][p][c, i, n], d = c*256+i*128+p
        x8c = x8[c * pc:(c + 1) * pc]
        xTt = np.ascontiguousarray(
            x8c.reshape(NG, GRP * P, NCH, 2, P).transpose(0, 4, 2, 3, 1)
               .reshape(NG, P, NCH * 2 * GRP * P))
        in_maps.append({
            "x": xt,
            "xT": xTt,
            "rel": np.ascontiguousarray(rel.T),
            "w1": w1q,
            "w2": w2q,
            "b1": b1q,
            "iota": iota,
        })
        meta.append(bases)

    _install_ntff_hook()
    _install_tile_compat()
    _patch_sim_dma_cost()
    from concourse.bass_utils import run_bass_kernel_spmd

    nc = _build_kernel(NT, windows, b2f + 25.0)
    _split_multi_waits(nc)

    trace = os.environ.get("KERNEL_TRACE", "") == "1"
    res = run_bass_kernel_spmd(nc, in_maps, list(range(N_CORES)), trace=trace)
    if trace and res.exec_time_ns:
        print(f"[kernel] HW exec time: {res.exec_time_ns} ns", file=sys.stderr)
        kernel.last_exec_time_ns = res.exec_time_ns

    # host unshard: accumulate windows, build denominators from e, divide
    u_sum = np.zeros((NUM_SEG, DIM), dtype=np.float64)
    e_full = np.empty(N, dtype=np.float64)
    for c in range(N_CORES):
        r = res.results[c]
        e_full[c * pc:(c + 1) * pc] = r["e"].T.reshape(-1).astype(np.float64)
        for w in range(len(windows)):
            base = meta[c][w]
            hi = min(base + P, NUM_SEG)
            u_sum[base:hi] += r["u"][w][:hi - base]
    denom = np.zeros(NUM_SEG, dtype=np.float64)
    np.add.at(denom, batch64, e_full)
    s_max = float(np.log(max(e_full.max(), 1e-30)))
    out = u_sum / (denom + 1e-8 * np.exp(s_max))[:, None]
    return out.astype(np.float32)


kernel.last_exec_time_ns = None


# revision 17
# speedup vs baseline: 1.0240x; 1.0240x over previous
"""nn_AttentionPool Trainium2 kernel (fp8-DoubleRow MLP + fp16 scatter).

kernel(x, batch, W1, b1, W2, b2) -> np.ndarray [2048, 1024] float32

Strategy (8 NeuronCores, SPMD, data-parallel over node rows; batch is
sorted so each core covers a contiguous segment range):
  - Host ships per core: x twice — dim-major fp8-e4m3 (MLP moving
    operand, DoubleRow-paired over the 1024 contraction) and node-major
    fp16 (scatter moving operand; fp16 because scatter errors hit the
    output directly).
  - Per 4-tile group (512 nodes) on device:
      PE:  hT[hh] += W1[:,c,:,hh]^T @ xT[:,c]   (fp8 DoubleRow, K=256/mm)
      ACT: thT = tanh(hT + b1)                  (fp16 out)
      PE:  s[i] += thT_slice^T @ w2             (fp16, N=1, FWL loads)
      DVE: sb = s - 25     (bias for the masked exp)
      DVE: mask[i] = (iota == rel)*(b2+25)      (0 or b2+25)
      ACT: A[i] = exp(mask + sb) -> fp16 one-hot row weights
           (match -> exp(s+b2), miss -> exp(s-25) ~ 0 in fp16)
      DVE: e[:,t] = rowsum(A)  (exactly the weights used in the scatter)
      PE:  u_win += A^T @ x    (fp16, 2x N=512, into static node-window
           PSUM accumulators; 3 windows of ~86 tiles per core)
  - Deep software pipeline (score 2 groups behind the MLP, scatter 4
    behind) so the tanh->s->exp cross-engine chain never starves the PE.
  - Host: accumulates window outputs by true segment base, builds
    denominators from e, divides (reference epsilon semantics).
Max-shift note: s in [-1.2, 1.2] for this model so unshifted exp is safe;
softmax normalization cancels any constant shift.
"""
import os
import sys
import types

import numpy as np

P = 128
DIM = 1024
HID = 256
GRP = 4            # 128-node tiles per group
N_CORES = 8
NUM_SEG = 2048
NCH = 4            # DoubleRow contraction chunks of 256 over DIM

# ---------------------------------------------------------------------------
# environment compat (axon-tunneled trn2 + this walrus build)
# ---------------------------------------------------------------------------

def _install_ntff_hook():
    """antenv.axon_hooks is absent in this image; reconstruct it so
    trace=True (KERNEL_TRACE=1) can profile. Harmless if unused."""
    if "antenv.axon_hooks" in sys.modules:
        return
    m = types.ModuleType("antenv.axon_hooks")
    m._hook = None
    m.set_axon_ntff_profile_hook = lambda h: setattr(m, "_hook", h)
    m.get_axon_ntff_profile_hook = lambda: m._hook
    sys.modules["antenv.axon_hooks"] = m
    try:
        from trn_agent_boot.trn_boot import _ntff_profile_via_ctypes
        m.set_axon_ntff_profile_hook(
            _ntff_profile_via_ctypes("/opt/axon/libaxon_pjrt.so"))
    except Exception:
        pass


def _install_tile_compat():
    """This walrus accepts at most ONE sem wait per instruction; Tile's exit
    drain carries one per live proc. Patch the drain to spread waits."""
    from concourse import mybir
    from concourse.tile import TileContext, ScopedClock

    if getattr(TileContext, "_attnpool_patched", False):
        return

    def _patched(self, tick_clock, wait_clock):
        drain_inst = self.nc.sync.drain()
        wait_clock.add_sem_waits(
            drain_inst.ins, ScopedClock({None: tick_clock.global_clock}))
        si = drain_inst.ins.sync_info
        waits = list(si.on_wait or [])
        if len(waits) > 1:
            si.on_wait = waits[:1]
            for i, w in enumerate(waits[1:]):
                nop = self.nc.sync.nop(nofuse=True, hint=f"tailwait{i}")
                nop.ins.sync_info = mybir.SyncInfo(on_wait=[w], on_update=[])
        self.nc.all_engine_barrier()
        popped = self.nc._tile_sem_poison_stack.pop()
        assert popped is self._sem_poison
        self.nc.clear_and_free_semaphores(list(self.sems.allocated().values()))
        self.nc.all_engine_barrier()

    TileContext._drain_and_barrier = _patched
    TileContext._attnpool_patched = True


def _patch_sim_dma_cost():
    """The Tile scheduler chooses instruction order from a cost-model sim.
    Its DMA model (~330GB/s per dma_start, 1.7us init) makes next-group
    x tiles look perpetually not-ready, so the greedy scheduler collapses
    the software pipeline into a serial per-group chain (PE idles ~1.5us
    per group waiting on the tanh->s->exp chain). Model DMA as 4x faster
    for scheduling only — real DMAs are deeply double-buffered, so the
    emitted lags then survive into the schedule. HW execution is
    unaffected (this spec feeds only the scheduling sim)."""
    from concourse import hw_specs
    if not getattr(hw_specs.TRN2Spec, "_attnpool_dma_patched", False):
        hw_specs.TRN2Spec.DMA_CYCLE = hw_specs.TRN2Spec.DMA_CYCLE / 4.0
        hw_specs.TRN2Spec._attnpool_dma_patched = True


def _split_multi_waits(nc):
    """Post-pass: hoist extra sem waits onto single-wait NOPs."""
    from concourse import mybir
    n = 0
    for f in nc.m.functions:
        for blk in f.blocks:
            new = []
            for inst in blk.instructions:
                si = inst.sync_info
                waits = list(si.on_wait or []) if si else []
                if len(waits) > 1:
                    for w in waits[:-1]:
                        n += 1
                        nop = mybir.InstNoOp(name=f"I-waitsplit{n}", ins=[], outs=[])
                        nop.engine = inst.engine
                        nop.sync_info = mybir.SyncInfo(on_wait=[w], on_update=[])
                        new.append(nop)
                    si.on_wait = waits[-1:]
                new.append(inst)
            blk.instructions = new


# ---------------------------------------------------------------------------
# device program
# ---------------------------------------------------------------------------

def _build_kernel(NT, windows, b2_plus_25):
    """windows: list of (a, b) tile ranges (128-node units)."""
    from concourse import bass, mybir
    import concourse.tile as tile

    f32 = mybir.dt.float32
    fp16 = mybir.dt.float16
    fp8 = mybir.dt.float8e4
    DR = mybir.MatmulPerfMode.DoubleRow

    nc = bass.Bass()
    NW = len(windows)
    NG = NT // GRP

    # x: node-major fp16 for scatter. [g][p][(t, d)], node = g*512+t*128+p
    x_in = nc.declare_dram_parameter("x", [NG, P, GRP * DIM], fp16,
                                     isOutput=False)
    # xT: dim-major fp8 for MLP. [g][p][c(4), i(2), n(512)], d = c*256+i*128+p
    xT_in = nc.declare_dram_parameter("xT", [NG, P, NCH * 2 * GRP * P], fp8,
                                      isOutput=False)
    rel_in = nc.declare_dram_parameter("rel", [P, NT], f32, isOutput=False)
    # w1: [p][c(4), i(2), h(256)] = W1[c*256 + i*128 + p, h]
    w1_in = nc.declare_dram_parameter("w1", [P, NCH * 2 * HID], fp8,
                                      isOutput=False)
    # w2: [p][hh(2)] = W2[hh*128 + p]
    w2_in = nc.declare_dram_parameter("w2", [P, 2], fp16, isOutput=False)
    b1_in = nc.declare_dram_parameter("b1", [P, 2], f32, isOutput=False)
    iota_in = nc.declare_dram_parameter("iota", [P, P], f32, isOutput=False)
    u_out = nc.declare_dram_parameter("u", [NW, P, DIM], f32, isOutput=True)
    e_out = nc.declare_dram_parameter("e", [P, NT], f32, isOutput=True)

    win_start = {a: w for w, (a, b) in enumerate(windows)}
    win_end = {b - 1: w for w, (a, b) in enumerate(windows)}
    tile_win = {}
    for w, (a, b) in enumerate(windows):
        for t in range(a, b):
            tile_win[t] = w

    with tile.TileContext(nc) as tc:
        with tc.tile_pool(name="const", bufs=1) as const, \
             tc.tile_pool(name="xpool", bufs=12) as xpool, \
             tc.tile_pool(name="xtpool", bufs=10) as xtpool, \
             tc.tile_pool(name="thpool", bufs=4) as thpool, \
             tc.tile_pool(name="apool", bufs=16) as apool, \
             tc.tile_pool(name="mpool", bufs=5) as mpool, \
             tc.tile_pool(name="spool", bufs=3) as spool, \
             tc.tile_pool(name="opool", bufs=2) as opool, \
             tc.tile_pool(name="pp_h", bufs=2, space="PSUM") as pp_h, \
             tc.tile_pool(name="pp_s", bufs=2, space="PSUM") as pp_s, \
             tc.tile_pool(name="pp_u", bufs=1, space="PSUM") as pp_u:

            w1t = const.tile([P, NCH, 2, HID], fp8)
            nc.sync.dma_start(out=w1t[:].rearrange("p c i h -> p (c i h)"),
                              in_=w1_in[:])
            w2t = const.tile([P, 2], fp16)
            nc.sync.dma_start(out=w2t[:], in_=w2_in[:])
            b1t = const.tile([P, 2], f32)
            nc.sync.dma_start(out=b1t[:], in_=b1_in[:])
            iota = const.tile([P, P], f32)
            nc.sync.dma_start(out=iota[:], in_=iota_in[:])
            relt = const.tile([P, NT], f32)
            nc.sync.dma_start(out=relt[:], in_=rel_in[:])
            e_stage = const.tile([P, NT], f32)

            state = {}
            ugroups = {}

            def emit_mlp(g):
                t0 = g * GRP
                xTg = xtpool.tile([P, NCH, 2, GRP * P], fp8, tag="xTg")
                nc.sync.dma_start(
                    out=xTg[:].rearrange("p c i n -> p (c i n)"), in_=xT_in[g])
                xg = xpool.tile([P, GRP, DIM], fp16, tag="xg")
                nc.sync.dma_start(
                    out=xg[:].rearrange("p t d -> p (t d)"), in_=x_in[g])
                mg = mpool.tile([P, GRP, P], f32, tag="mg")
                for i in range(GRP):
                    nc.vector.tensor_scalar(
                        out=mg[:, i],
                        in0=iota[:],
                        scalar1=relt[:, t0 + i:t0 + i + 1],
                        scalar2=float(b2_plus_25),
                        op0=mybir.AluOpType.is_equal,
                        op1=mybir.AluOpType.mult)

                hts = []
                for hh in range(2):
                    hTp = pp_h.tile([P, GRP * P], f32, tag=f"hT{hh}",
                                    name=f"hTp{hh}")
                    for c in range(NCH):
                        nc.tensor.matmul(
                            hTp[:],
                            lhsT=w1t[:, c, :, hh * P:(hh + 1) * P],
                            rhs=xTg[:, c],
                            start=(c == 0), stop=(c == NCH - 1),
                            perf_mode=DR)
                    hts.append(hTp)

                thT = thpool.tile([P, 2, GRP * P], fp16, tag="thT")
                for hh in range(2):
                    nc.scalar.activation(
                        thT[:, hh], hts[hh][:],
                        mybir.ActivationFunctionType.Tanh,
                        bias=b1t[:, hh:hh + 1])
                state[g] = {"xg": xg, "thT": thT, "mg": mg}

            def emit_s(g):
                st = state[g]
                sp = pp_s.tile([P, GRP], mybir.dt.float32, tag="sp")
                for i in range(GRP):
                    for hh in range(2):
                        nc.tensor.matmul(
                            sp[:, i:i + 1],
                            lhsT=st["thT"][:, hh, i * P:(i + 1) * P],
                            rhs=w2t[:, hh:hh + 1],
                            start=(hh == 0), stop=(hh == 1))
                sb = spool.tile([P, GRP], mybir.dt.float32, tag="sb")
                nc.vector.tensor_scalar(
                    out=sb[:], in0=sp[:], scalar1=-25.0, scalar2=None,
                    op0=mybir.AluOpType.add, op1=mybir.AluOpType.bypass)
                As = []
                for i in range(GRP):
                    t = g * GRP + i
                    A = apool.tile([P, P], fp16, tag="A")
                    nc.scalar.activation(
                        A[:], st["mg"][:, i],
                        mybir.ActivationFunctionType.Exp,
                        bias=sb[:, i:i + 1])
                    As.append(A)
                    nc.vector.reduce_sum(
                        e_stage[:, t:t + 1], A[:], axis=mybir.AxisListType.X)
                st["As"] = As

            def emit_scatter(g):
                st = state[g]
                for i in range(GRP):
                    t = g * GRP + i
                    xt = st["xg"][:, i]
                    w = tile_win[t]
                    if t in win_start:
                        uwin = pp_u.tile([P, DIM], mybir.dt.float32,
                                         tag="uwin")
                        ugroups[w] = uwin
                    up = ugroups[w]
                    for half in range(2):
                        nc.tensor.matmul(
                            up[:, half * 512:(half + 1) * 512],
                            lhsT=st["As"][i][:],
                            rhs=xt[:, half * 512:(half + 1) * 512],
                            start=(t in win_start), stop=(t in win_end))
                    if t in win_end:
                        us = opool.tile([P, DIM], mybir.dt.float32, tag="us")
                        nc.vector.tensor_copy(us[:, 0:512], up[:, 0:512])
                        nc.scalar.copy(us[:, 512:1024], up[:, 512:1024])
                        nc.sync.dma_start(out=u_out[w], in_=us[:])
                del state[g]

            S_LAG, SC_LAG = 2, 4
            for g in range(NG):
                emit_mlp(g)
                if g >= S_LAG:
                    emit_s(g - S_LAG)
                if g >= SC_LAG:
                    emit_scatter(g - SC_LAG)
            for g in range(NG - S_LAG, NG):
                emit_s(g)
            for g in range(NG - SC_LAG, NG):
                emit_scatter(g)

            nc.sync.dma_start(out=e_out[:], in_=e_stage[:])


    return nc


# ---------------------------------------------------------------------------
# host wrapper
# ---------------------------------------------------------------------------

def _make_windows(n, nw):
    base, rem = divmod(n, nw)
    sizes = [base + (1 if i < rem else 0) for i in range(nw)]
    out, a = [], 0
    for s in sizes:
        out.append((a, a + s))
        a += s
    return out


def _reference_numpy(x, batch, W1, b1, W2, b2):
    """Fallback for inputs outside this kernel's structural assumptions."""
    h = np.tanh(x.astype(np.float64) @ W1.astype(np.float64) + b1)
    s = (h @ W2.astype(np.float64) + b2).ravel()
    e = np.exp(s - s.max())
    denom = np.zeros(NUM_SEG, dtype=np.float64)
    np.add.at(denom, batch, e)
    attn = e / (denom[batch] + 1e-8)
    out = np.zeros((NUM_SEG, x.shape[1]), dtype=np.float64)
    np.add.at(out, batch, attn[:, None] * x.astype(np.float64))
    return out.astype(np.float32)


def kernel(x, batch, W1, b1, W2, b2):
    import ml_dtypes
    fp8 = ml_dtypes.float8_e4m3

    x = np.ascontiguousarray(np.asarray(x, dtype=np.float32))
    batch64 = np.asarray(batch).astype(np.int64)
    W1 = np.asarray(W1, dtype=np.float32)
    b1 = np.asarray(b1, dtype=np.float32)
    W2 = np.asarray(W2, dtype=np.float32)
    b2 = np.asarray(b2, dtype=np.float32)

    N = x.shape[0]
    pc = N // N_CORES
    NT = pc // P
    NG = NT // GRP if NT else 0

    ok = (N == N_CORES * pc and pc == NT * P and NT % GRP == 0
          and x.shape[1] == DIM and W1.shape == (DIM, HID)
          and np.all(batch64[:-1] <= batch64[1:])
          and batch64.min() >= 0 and batch64.max() < NUM_SEG)
    if not ok:
        return _reference_numpy(x, batch64, W1, b1, W2, b2)

    windows = _make_windows(NT, 3)
    iota = np.tile(np.arange(P, dtype=np.float32), (P, 1))
    b2f = float(b2.reshape(-1)[0])

    # weights, shared across cores
    w1q = np.ascontiguousarray(
        W1.astype(fp8).reshape(NCH, 2, P, HID).transpose(2, 0, 1, 3)
        .reshape(P, NCH * 2 * HID))
    w2q = np.ascontiguousarray(W2.astype(np.float16).reshape(2, P).T)
    b1q = np.ascontiguousarray(b1.reshape(2, P).T)

    x8 = x.astype(fp8)

    in_maps, meta = [], []
    for c in range(N_CORES):
        bb = batch64[c * pc:(c + 1) * pc]
        rel = np.empty((NT, P), dtype=np.float32)
        bases = []
        for w, (a, b) in enumerate(windows):
            base = int(bb[a * P])
            bases.append(base)
            seg_rel = bb[a * P:b * P] - base
            if seg_rel.min() < 0 or seg_rel.max() >= P:
                return _reference_numpy(x, batch64, W1, b1, W2, b2)
            rel[a:b] = seg_rel.reshape(b - a, P).astype(np.float32)
        # node-major fp16 (scatter)
        x16 = x[c * pc:(c + 1) * pc].astype(np.float16)
        xt = np.ascontiguousarray(
            x16.reshape(NG, GRP, P, DIM).transpose(0, 2, 1, 3)
               .reshape(NG, P, GRP * DIM))
        # dim-major fp8 (MLP): [g][p][c, i, n], d = c*256+i*128+p
        x8c = x8[c * pc:(c + 1) * pc]
        xTt = np.ascontiguousarray(
            x8c.reshape(NG, GRP * P, NCH, 2, P).transpose(0, 4, 2, 3, 1)
               .reshape(NG, P, NCH * 2 * GRP * P))
        in_maps.append({
            "x": xt,
            "xT": xTt,
            "rel": np.ascontiguousarray(rel.T),
            "w1": w1q,
            "w2": w2q,
            "b1": b1q,
            "iota": iota,
        })
        meta.append(bases)

    _install_ntff_hook()
    _install_tile_compat()
    _patch_sim_dma_cost()
    from concourse.bass_utils import run_bass_kernel_spmd

    nc = _build_kernel(NT, windows, b2f + 25.0)
    _split_multi_waits(nc)

    trace = os.environ.get("KERNEL_TRACE", "") == "1"
    res = run_bass_kernel_spmd(nc, in_maps, list(range(N_CORES)), trace=trace)
    if trace and res.exec_time_ns:
        print(f"[kernel] HW exec time: {res.exec_time_ns} ns", file=sys.stderr)
        kernel.last_exec_time_ns = res.exec_time_ns

    # host unshard: accumulate windows, build denominators from e, divide
    u_sum = np.zeros((NUM_SEG, DIM), dtype=np.float64)
    e_full = np.empty(N, dtype=np.float64)
    for c in range(N_CORES):
        r = res.results[c]
        e_full[c * pc:(c + 1) * pc] = r["e"].T.reshape(-1).astype(np.float64)
        for w in range(len(windows)):
            base = meta[c][w]
            hi = min(base + P, NUM_SEG)
            u_sum[base:hi] += r["u"][w][:hi - base]
    denom = np.zeros(NUM_SEG, dtype=np.float64)
    np.add.at(denom, batch64, e_full)
    s_max = float(np.log(max(e_full.max(), 1e-30)))
    out = u_sum / (denom + 1e-8 * np.exp(s_max))[:, None]
    return out.astype(np.float32)


kernel.last_exec_time_ns = None
